# revision 2
# baseline (speedup 1.0000x reference)
"""MoE all-reduce + RMSNorm fused kernel for Trainium2 (8 NeuronCores), v3.

    expert_reduction = einsum("eth,et->th", active_experts_token_input, scale_input)
    output_residual  = expert_reduction + token_input + residual
    hidden_states    = output_residual * rsqrt(mean(output_residual^2, -1) + 1e-5) * norm_weight

Tokens sharded across cores (no collectives). All large tensors move as fp16
(max-rel error ~1.4e-3, gate 2e-2); tok+res are pre-summed host-side (`base`).

v3 moves the expert MACs off DVE onto the PE array: per-token scaling is a
matmul against a host-packed diagonal matrix diag(scale[e, chunk]) in fp16,
accumulated over the 8 experts in PSUM fp32. (DVE per-partition-scalar ops
run at 1x — no 2x/4x perf modes — so v2 was DVE-bound at 345us.) DVE is left
with the PSUM+base merge and the nw multiply (2x fp16 mode); ACT does the
mean-square, the y2 per-token scale, and issues stores on its own HWDGE ring
so the SP ring carries loads only.

DMA layout: `a` packed host-side as [chunk][half][p][e][hh] so each
(chunk, half) slab is one contiguous 4 MiB DMA with 32 KiB per-partition runs.

VARIANT=int8: `a` stored int8 in HBM (per (e,t)-row absmax quant, dequant
folded into the diag), SWDGE cast-on-load int8->fp16 halves `a` read traffic.
"""

import os
import sys

import numpy as np

try:
    import concourse  # noqa: F401
except ImportError:
    sys.path.insert(0, "/opt/trn_rl_repo")

E, T, H = 8, 8192, 4096
N_CORES = 8
T_CORE = T // N_CORES   # 1024 tokens per core
P = 128                 # SBUF partitions = tokens per chunk
N_CHUNKS = T_CORE // P  # 8
HH = H // 2             # half-row processed per pipeline step
NB = HH // 512          # PSUM banks per half
EPS = 1e-5

A_INT8 = os.environ.get("VARIANT", "int8") == "int8"

_CACHE = {}


def _build_program():
    from contextlib import ExitStack

    import concourse.bass as bass  # noqa: F401
    from concourse import bacc, mybir, tile

    f32 = mybir.dt.float32
    f16 = mybir.dt.float16
    i8 = mybir.dt.int8
    mult = mybir.AluOpType.mult
    add = mybir.AluOpType.add
    Square = mybir.ActivationFunctionType.Square
    Sqrt = mybir.ActivationFunctionType.Sqrt
    Copy = mybir.ActivationFunctionType.Copy

    nc = bacc.Bacc(
        "TRN2",
        target_bir_lowering=False,
        debug=False,
        enable_asserts=False,
        num_devices=N_CORES,
    )

    a_dt = i8 if A_INT8 else f16
    # packed host-side: [chunk, half, p, e, hh]
    a = nc.dram_tensor("a_in", [N_CHUNKS, 2, P, E, HH], a_dt, kind="ExternalInput").ap()
    base = nc.dram_tensor("base_in", [T_CORE, H], f16, kind="ExternalInput").ap()
    # host-packed diag matrices: col (c*E+e)*P + m holds scale[e, c*128+p]*(p==m)
    dg = nc.dram_tensor(
        "dg_in", [P, N_CHUNKS * E * P], f16, kind="ExternalInput"
    ).ap()
    nw = nc.dram_tensor("nw_in", [P, H], f16, kind="ExternalInput").ap()
    hid_out = nc.dram_tensor("hid_out", [T_CORE, H], f16, kind="ExternalOutput").ap()
    ores_out = nc.dram_tensor("ores_out", [T_CORE, H], f16, kind="ExternalOutput").ap()

    with tile.TileContext(nc) as tc, ExitStack() as ctx:
        nw_pool = ctx.enter_context(tc.tile_pool(name="nw", bufs=1))
        a_pool = ctx.enter_context(tc.tile_pool(name="a", bufs=2))
        tr_pool = ctx.enter_context(tc.tile_pool(name="tr", bufs=2))
        acc_pool = ctx.enter_context(tc.tile_pool(name="acc", bufs=2))
        hid_pool = ctx.enter_context(tc.tile_pool(name="hid", bufs=2))
        tmp_pool = ctx.enter_context(tc.tile_pool(name="tmp", bufs=2))
        st_pool = ctx.enter_context(tc.tile_pool(name="st", bufs=2))
        ps_pool = ctx.enter_context(tc.tile_pool(name="ps", bufs=2, space="PSUM"))

        # one-time preloads on the SWDGE path (keep the HWDGE load FIFO clean)
        dg_t = nw_pool.tile([P, N_CHUNKS * E * P], f16, tag="dg")
        nc.gpsimd.dma_start(out=dg_t[:], in_=dg[:, :])
        nw_t = nw_pool.tile([P, H], f16)
        nc.gpsimd.dma_start(out=nw_t[:], in_=nw[:, :])

        zero_t = nw_pool.tile([P, 1], f32, tag="zero")
        nc.vector.memset(zero_t[:], 0.0)
        eps_t = nw_pool.tile([P, 1], f32, tag="eps")
        nc.vector.memset(eps_t[:], EPS)

        # dummy target for the Square activation (only accum_out is used)
        sq_t = nw_pool.tile([P, HH], f16, tag="sq")

        for c in range(N_CHUNKS):
            t0 = c * P
            base_t = tr_pool.tile([P, H], f16, tag="tr")
            nc.sync.dma_start(out=base_t[:], in_=base[t0 : t0 + P, :])

            acc_t = acc_pool.tile([P, H], f16)
            var_parts = []
            for s in range(2):
                cols = slice(s * HH, (s + 1) * HH)
                a_t = a_pool.tile([P, E * HH], f16, tag="a_t")
                if A_INT8:
                    nc.gpsimd.dma_start(out=a_t[:], in_=a[c, s])
                else:
                    nc.sync.dma_start(out=a_t[:], in_=a[c, s])

                ps_t = ps_pool.tile([P, HH], f32, tag="ps")
                for b in range(NB):
                    for e in range(E):
                        di = (c * E + e) * P
                        nc.tensor.matmul(
                            ps_t[:, b * 512 : (b + 1) * 512],
                            dg_t[:, di : di + P],
                            a_t[:, e * HH + b * 512 : e * HH + (b + 1) * 512],
                            start=(e == 0),
                            stop=(e == E - 1),
                        )
                nc.vector.tensor_tensor(
                    out=acc_t[:, cols], in0=ps_t[:], in1=base_t[:, cols], op=add
                )
                nc.scalar.dma_start(
                    out=ores_out[t0 : t0 + P, cols], in_=acc_t[:, cols]
                )

                # partial mean-square on ACT: sum(Square(acc/64)) = sum(acc^2)/4096
                var_t = st_pool.tile([P, 1], f32, tag="var")
                nc.scalar.activation(
                    out=sq_t[:], in_=acc_t[:, cols], func=Square,
                    scale=1.0 / 64.0, bias=zero_t[:, 0:1], accum_out=var_t[:],
                )
                var_parts.append(var_t)

            vsum_t = st_pool.tile([P, 1], f32, tag="vsum")
            nc.vector.tensor_tensor(
                out=vsum_t[:], in0=var_parts[0][:], in1=var_parts[1][:], op=add
            )
            # rsqrt(var + eps): ACT Sqrt seed + DVE reciprocal + 1 Newton step
            std_t = st_pool.tile([P, 1], f32, tag="std")
            nc.scalar.activation(
                out=std_t[:], in_=vsum_t[:], func=Sqrt, bias=eps_t[:, 0:1]
            )
            y_t = st_pool.tile([P, 1], f32, tag="y")
            nc.vector.reciprocal(out=y_t[:], in_=std_t[:])
            x_t = st_pool.tile([P, 1], f32, tag="x")
            nc.vector.tensor_scalar_add(x_t[:], vsum_t[:], EPS)
            t_t = st_pool.tile([P, 1], f32, tag="t")
            nc.vector.tensor_tensor(out=t_t[:], in0=y_t[:], in1=y_t[:], op=mult)
            nc.vector.tensor_tensor(out=t_t[:], in0=t_t[:], in1=x_t[:], op=mult)
            h_t = st_pool.tile([P, 1], f32, tag="h")
            nc.vector.tensor_scalar(
                out=h_t[:], in0=t_t[:], scalar1=-0.5, scalar2=1.5, op0=mult, op1=add
            )
            y2_t = st_pool.tile([P, 1], f32, tag="y2")
            nc.vector.tensor_tensor(out=y2_t[:], in0=y_t[:], in1=h_t[:], op=mult)

            hid_t = hid_pool.tile([P, H], f16)
            for s in range(2):
                cols = slice(s * HH, (s + 1) * HH)
                # ACT: tmp = acc * y2 (per-token scale), then DVE 2x: *nw
                tmp_t = tmp_pool.tile([P, HH], f16, tag="tmp")
                nc.scalar.activation(
                    out=tmp_t[:], in_=acc_t[:, cols], func=Copy,
                    scale=y2_t[:, 0:1],
                )
                nc.vector.tensor_tensor(
                    out=hid_t[:, cols], in0=tmp_t[:], in1=nw_t[:, cols], op=mult
                )
            nc.scalar.dma_start(out=hid_out[t0 : t0 + P, :], in_=hid_t[:])

    nc.compile()
    return nc


def _get_program():
    if "nc" not in _CACHE:
        _CACHE["nc"] = _build_program()
    return _CACHE["nc"]


def _make_in_maps(residual, norm_weight, scale_input, active, token_input):
    nw16 = np.asarray(norm_weight, np.float16)
    nw_b = np.ascontiguousarray(np.broadcast_to(nw16, (P, H)))
    base16 = (np.asarray(residual, np.float32) + np.asarray(token_input, np.float32)
              ).astype(np.float16)

    if A_INT8:
        # per (e,t)-row absmax int8 quantization; dequant folded into the diag
        absmax = np.abs(active).max(axis=2)                      # [E, T]
        r = np.maximum(absmax, 1e-30) / 127.0                    # [E, T]
        q = np.clip(np.rint(active / r[:, :, None]), -127, 127).astype(np.int8)
        sc_eff = np.asarray(scale_input, np.float32) * r
        a_src = q
    else:
        a_src = np.asarray(active, np.float16)
        sc_eff = np.asarray(scale_input, np.float32)

    ar = np.arange(P)
    in_maps = []
    for c in range(N_CORES):
        lo, hi = c * T_CORE, (c + 1) * T_CORE
        # [e, chunk*P+p, half*HH+hh] -> [chunk, half, p, e, hh]
        ap = np.ascontiguousarray(
            a_src[:, lo:hi, :]
            .reshape(E, N_CHUNKS, P, 2, HH)
            .transpose(1, 3, 2, 0, 4)
        )
        # scales [P, c*E+e] for this core
        scv = (
            sc_eff[:, lo:hi]
            .reshape(E, N_CHUNKS, P)
            .transpose(2, 1, 0)
            .reshape(P, N_CHUNKS * E)
            .astype(np.float16)
        )
        # diag matrices [idx, p, m]: nonzero only at p==m
        dgm = np.zeros((N_CHUNKS * E, P, P), np.float16)
        dgm[:, ar, ar] = scv.T
        dg = np.ascontiguousarray(
            dgm.transpose(1, 0, 2).reshape(P, N_CHUNKS * E * P)
        )
        in_maps.append(
            {
                "a_in": ap,
                "base_in": np.ascontiguousarray(base16[lo:hi]),
                "dg_in": dg,
                "nw_in": nw_b,
            }
        )
    return in_maps


def _ensure_ntff_hook():
    """Register the axon NTFF profiling hook if the image's antenv lacks it."""
    import types

    name = "antenv.axon_hooks"
    if name in sys.modules:
        return
    try:
        import antenv.axon_hooks  # noqa: F401

        return
    except ImportError:
        pass
    mod = types.ModuleType(name)
    mod._hook = None
    mod.set_axon_ntff_profile_hook = lambda h: setattr(mod, "_hook", h)
    mod.get_axon_ntff_profile_hook = lambda: mod._hook
    sys.modules[name] = mod
    try:
        from trn_agent_boot.trn_boot import _ntff_profile_via_ctypes

        h = _ntff_profile_via_ctypes("/opt/axon/libaxon_pjrt.so")
        if h is not None:
            mod._hook = h
    except Exception:
        pass


def kernel(
    residual,
    norm_weight,
    scale_input,
    active_experts_token_input,
    token_input,
    device_num_experts,
    _trace=False,
):
    if _trace:
        _ensure_ntff_hook()
    from concourse.bass_utils import run_bass_kernel_spmd

    assert int(device_num_experts) == E
    residual = np.asarray(residual, np.float32)
    norm_weight = np.asarray(norm_weight, np.float32)
    scale_input = np.asarray(scale_input, np.float32)
    active = np.asarray(active_experts_token_input, np.float32)
    token_input = np.asarray(token_input, np.float32)

    nc = _get_program()
    in_maps = _make_in_maps(residual, norm_weight, scale_input, active, token_input)
    r = run_bass_kernel_spmd(nc, in_maps, list(range(N_CORES)), trace=_trace)
    hidden = np.concatenate(
        [np.asarray(r.results[c]["hid_out"], np.float32) for c in range(N_CORES)],
        axis=0,
    )
    outres = np.concatenate(
        [np.asarray(r.results[c]["ores_out"], np.float32) for c in range(N_CORES)],
        axis=0,
    )
    if _trace:
        _CACHE["last_result"] = r
    return hidden, outres


# revision 3
# speedup vs baseline: 1.0254x; 1.0254x over previous
"""MoE all-reduce + RMSNorm fused kernel for Trainium2 (8 NeuronCores), v3.

    expert_reduction = einsum("eth,et->th", active_experts_token_input, scale_input)
    output_residual  = expert_reduction + token_input + residual
    hidden_states    = output_residual * rsqrt(mean(output_residual^2, -1) + 1e-5) * norm_weight

Tokens sharded across cores (no collectives). All large tensors move as fp16
(max-rel error ~1.4e-3, gate 2e-2); tok+res are pre-summed host-side (`base`).

v3 moves the expert MACs off DVE onto the PE array: per-token scaling is a
matmul against a host-packed diagonal matrix diag(scale[e, chunk]) in fp16,
accumulated over the 8 experts in PSUM fp32. (DVE per-partition-scalar ops
run at 1x — no 2x/4x perf modes — so v2 was DVE-bound at 345us.) DVE is left
with the PSUM+base merge and the nw multiply (2x fp16 mode); ACT does the
mean-square, the y2 per-token scale, and issues stores on its own HWDGE ring
so the SP ring carries loads only.

DMA layout: `a` packed host-side as [chunk][half][p][e][hh] so each
(chunk, half) slab is one contiguous 4 MiB DMA with 32 KiB per-partition runs.

VARIANT=int8: `a` stored int8 in HBM (per (e,t)-row absmax quant, dequant
folded into the diag), SWDGE cast-on-load int8->fp16 halves `a` read traffic.
"""

import os
import sys

import numpy as np

try:
    import concourse  # noqa: F401
except ImportError:
    sys.path.insert(0, "/opt/trn_rl_repo")

E, T, H = 8, 8192, 4096
N_CORES = 8
T_CORE = T // N_CORES   # 1024 tokens per core
P = 128                 # SBUF partitions = tokens per chunk
N_CHUNKS = T_CORE // P  # 8
HH = H // 2             # half-row processed per pipeline step
NB = HH // 512          # PSUM banks per half
EPS = 1e-5

A_INT8 = os.environ.get("VARIANT", "int8") == "int8"

_CACHE = {}


def _build_program():
    from contextlib import ExitStack

    import concourse.bass as bass  # noqa: F401
    from concourse import bacc, mybir, tile

    f32 = mybir.dt.float32
    f16 = mybir.dt.float16
    i8 = mybir.dt.int8
    mult = mybir.AluOpType.mult
    add = mybir.AluOpType.add
    Square = mybir.ActivationFunctionType.Square
    Sqrt = mybir.ActivationFunctionType.Sqrt
    Copy = mybir.ActivationFunctionType.Copy

    nc = bacc.Bacc(
        "TRN2",
        target_bir_lowering=False,
        debug=False,
        enable_asserts=False,
        num_devices=N_CORES,
    )

    a_dt = i8 if A_INT8 else f16
    # packed host-side: [chunk, half, p, e, hh]
    a = nc.dram_tensor("a_in", [N_CHUNKS, 2, P, E, HH], a_dt, kind="ExternalInput").ap()
    base = nc.dram_tensor("base_in", [T_CORE, H], f16, kind="ExternalInput").ap()
    # host-packed diag matrices: col (c*E+e)*P + m holds scale[e, c*128+p]*(p==m)
    dg = nc.dram_tensor(
        "dg_in", [P, N_CHUNKS * E * P], f16, kind="ExternalInput"
    ).ap()
    nw = nc.dram_tensor("nw_in", [P, H], f16, kind="ExternalInput").ap()
    hid_out = nc.dram_tensor("hid_out", [T_CORE, H], f16, kind="ExternalOutput").ap()
    ores_out = nc.dram_tensor("ores_out", [T_CORE, H], f16, kind="ExternalOutput").ap()

    with tile.TileContext(nc) as tc, ExitStack() as ctx:
        nw_pool = ctx.enter_context(tc.tile_pool(name="nw", bufs=1))
        a_pool = ctx.enter_context(tc.tile_pool(name="a", bufs=3))
        tr_pool = ctx.enter_context(tc.tile_pool(name="tr", bufs=2))
        acc_pool = ctx.enter_context(tc.tile_pool(name="acc", bufs=2))
        hid_pool = ctx.enter_context(tc.tile_pool(name="hid", bufs=2))
        tmp_pool = ctx.enter_context(tc.tile_pool(name="tmp", bufs=2))
        st_pool = ctx.enter_context(tc.tile_pool(name="st", bufs=2))
        ps_pool = ctx.enter_context(tc.tile_pool(name="ps", bufs=2, space="PSUM"))

        # one-time preloads on the SWDGE path (keep the HWDGE load FIFO clean)
        dg_t = nw_pool.tile([P, N_CHUNKS * E * P], f16, tag="dg")
        nc.scalar.dma_start(out=dg_t[:], in_=dg[:, :])
        nw_t = nw_pool.tile([P, H], f16)
        nc.scalar.dma_start(out=nw_t[:], in_=nw[:, :])

        zero_t = nw_pool.tile([P, 1], f32, tag="zero")
        nc.vector.memset(zero_t[:], 0.0)
        eps_t = nw_pool.tile([P, 1], f32, tag="eps")
        nc.vector.memset(eps_t[:], EPS)

        # dummy target for the Square activation (only accum_out is used)
        sq_t = nw_pool.tile([P, HH], f16, tag="sq")

        for c in range(N_CHUNKS):
            t0 = c * P
            base_t = tr_pool.tile([P, H], f16, tag="tr")
            nc.sync.dma_start(out=base_t[:], in_=base[t0 : t0 + P, :])

            acc_t = acc_pool.tile([P, H], f16)
            var_parts = []
            for s in range(2):
                cols = slice(s * HH, (s + 1) * HH)
                a_t = a_pool.tile([P, E * HH], f16, tag="a_t")
                eng = nc.gpsimd if A_INT8 else nc.sync
                if c == 0 and s == 0:
                    eng.dma_start(out=a_t[:, 0 : 4 * HH], in_=a[c, s, :, 0:4, :])
                    eng.dma_start(out=a_t[:, 4 * HH :], in_=a[c, s, :, 4:8, :])
                else:
                    eng.dma_start(out=a_t[:], in_=a[c, s])

                ps_t = ps_pool.tile([P, HH], f32, tag="ps")
                for b in range(NB):
                    for e in range(E):
                        di = (c * E + e) * P
                        nc.tensor.matmul(
                            ps_t[:, b * 512 : (b + 1) * 512],
                            dg_t[:, di : di + P],
                            a_t[:, e * HH + b * 512 : e * HH + (b + 1) * 512],
                            start=(e == 0),
                            stop=(e == E - 1),
                        )
                nc.vector.tensor_tensor(
                    out=acc_t[:, cols], in0=ps_t[:], in1=base_t[:, cols], op=add
                )
                nc.scalar.dma_start(
                    out=ores_out[t0 : t0 + P, cols], in_=acc_t[:, cols]
                )

                # partial mean-square on ACT: sum(Square(acc/64)) = sum(acc^2)/4096
                var_t = st_pool.tile([P, 1], f32, tag="var")
                nc.scalar.activation(
                    out=sq_t[:], in_=acc_t[:, cols], func=Square,
                    scale=1.0 / 64.0, bias=zero_t[:, 0:1], accum_out=var_t[:],
                )
                var_parts.append(var_t)

            vsum_t = st_pool.tile([P, 1], f32, tag="vsum")
            nc.vector.tensor_tensor(
                out=vsum_t[:], in0=var_parts[0][:], in1=var_parts[1][:], op=add
            )
            # rsqrt(var + eps): ACT Sqrt seed + DVE reciprocal + 1 Newton step
            std_t = st_pool.tile([P, 1], f32, tag="std")
            nc.scalar.activation(
                out=std_t[:], in_=vsum_t[:], func=Sqrt, bias=eps_t[:, 0:1]
            )
            y_t = st_pool.tile([P, 1], f32, tag="y")
            nc.vector.reciprocal(out=y_t[:], in_=std_t[:])
            x_t = st_pool.tile([P, 1], f32, tag="x")
            nc.vector.tensor_scalar_add(x_t[:], vsum_t[:], EPS)
            t_t = st_pool.tile([P, 1], f32, tag="t")
            nc.vector.tensor_tensor(out=t_t[:], in0=y_t[:], in1=y_t[:], op=mult)
            nc.vector.tensor_tensor(out=t_t[:], in0=t_t[:], in1=x_t[:], op=mult)
            h_t = st_pool.tile([P, 1], f32, tag="h")
            nc.vector.tensor_scalar(
                out=h_t[:], in0=t_t[:], scalar1=-0.5, scalar2=1.5, op0=mult, op1=add
            )
            y2_t = st_pool.tile([P, 1], f32, tag="y2")
            nc.vector.tensor_tensor(out=y2_t[:], in0=y_t[:], in1=h_t[:], op=mult)

            hid_t = hid_pool.tile([P, H], f16)
            for s in range(2):
                cols = slice(s * HH, (s + 1) * HH)
                # ACT: tmp = acc * y2 (per-token scale), then DVE 2x: *nw
                tmp_t = tmp_pool.tile([P, HH], f16, tag="tmp")
                nc.scalar.activation(
                    out=tmp_t[:], in_=acc_t[:, cols], func=Copy,
                    scale=y2_t[:, 0:1],
                )
                nc.vector.tensor_tensor(
                    out=hid_t[:, cols], in0=tmp_t[:], in1=nw_t[:, cols], op=mult
                )
                if c == N_CHUNKS - 1:
                    nc.scalar.dma_start(
                        out=hid_out[t0 : t0 + P, cols], in_=hid_t[:, cols]
                    )
            if c < N_CHUNKS - 1:
                nc.scalar.dma_start(out=hid_out[t0 : t0 + P, :], in_=hid_t[:])

    nc.compile()
    return nc


def _get_program():
    if "nc" not in _CACHE:
        _CACHE["nc"] = _build_program()
    return _CACHE["nc"]


def _make_in_maps(residual, norm_weight, scale_input, active, token_input):
    nw16 = np.asarray(norm_weight, np.float16)
    nw_b = np.ascontiguousarray(np.broadcast_to(nw16, (P, H)))
    base16 = (np.asarray(residual, np.float32) + np.asarray(token_input, np.float32)
              ).astype(np.float16)

    if A_INT8:
        # per (e,t)-row absmax int8 quantization; dequant folded into the diag
        absmax = np.abs(active).max(axis=2)                      # [E, T]
        r = np.maximum(absmax, 1e-30) / 127.0                    # [E, T]
        q = np.clip(np.rint(active / r[:, :, None]), -127, 127).astype(np.int8)
        sc_eff = np.asarray(scale_input, np.float32) * r
        a_src = q
    else:
        a_src = np.asarray(active, np.float16)
        sc_eff = np.asarray(scale_input, np.float32)

    ar = np.arange(P)
    in_maps = []
    for c in range(N_CORES):
        lo, hi = c * T_CORE, (c + 1) * T_CORE
        # [e, chunk*P+p, half*HH+hh] -> [chunk, half, p, e, hh]
        ap = np.ascontiguousarray(
            a_src[:, lo:hi, :]
            .reshape(E, N_CHUNKS, P, 2, HH)
            .transpose(1, 3, 2, 0, 4)
        )
        # scales [P, c*E+e] for this core
        scv = (
            sc_eff[:, lo:hi]
            .reshape(E, N_CHUNKS, P)
            .transpose(2, 1, 0)
            .reshape(P, N_CHUNKS * E)
            .astype(np.float16)
        )
        # diag matrices [idx, p, m]: nonzero only at p==m
        dgm = np.zeros((N_CHUNKS * E, P, P), np.float16)
        dgm[:, ar, ar] = scv.T
        dg = np.ascontiguousarray(
            dgm.transpose(1, 0, 2).reshape(P, N_CHUNKS * E * P)
        )
        in_maps.append(
            {
                "a_in": ap,
                "base_in": np.ascontiguousarray(base16[lo:hi]),
                "dg_in": dg,
                "nw_in": nw_b,
            }
        )
    return in_maps


def _ensure_ntff_hook():
    """Register the axon NTFF profiling hook if the image's antenv lacks it."""
    import types

    name = "antenv.axon_hooks"
    if name in sys.modules:
        return
    try:
        import antenv.axon_hooks  # noqa: F401

        return
    except ImportError:
        pass
    mod = types.ModuleType(name)
    mod._hook = None
    mod.set_axon_ntff_profile_hook = lambda h: setattr(mod, "_hook", h)
    mod.get_axon_ntff_profile_hook = lambda: mod._hook
    sys.modules[name] = mod
    try:
        from trn_agent_boot.trn_boot import _ntff_profile_via_ctypes

        h = _ntff_profile_via_ctypes("/opt/axon/libaxon_pjrt.so")
        if h is not None:
            mod._hook = h
    except Exception:
        pass


def kernel(
    residual,
    norm_weight,
    scale_input,
    active_experts_token_input,
    token_input,
    device_num_experts,
    _trace=False,
):
    if _trace:
        _ensure_ntff_hook()
    from concourse.bass_utils import run_bass_kernel_spmd

    assert int(device_num_experts) == E
    residual = np.asarray(residual, np.float32)
    norm_weight = np.asarray(norm_weight, np.float32)
    scale_input = np.asarray(scale_input, np.float32)
    active = np.asarray(active_experts_token_input, np.float32)
    token_input = np.asarray(token_input, np.float32)

    nc = _get_program()
    in_maps = _make_in_maps(residual, norm_weight, scale_input, active, token_input)
    r = run_bass_kernel_spmd(nc, in_maps, list(range(N_CORES)), trace=_trace)
    hidden = np.concatenate(
        [np.asarray(r.results[c]["hid_out"], np.float32) for c in range(N_CORES)],
        axis=0,
    )
    outres = np.concatenate(
        [np.asarray(r.results[c]["ores_out"], np.float32) for c in range(N_CORES)],
        axis=0,
    )
    if _trace:
        _CACHE["last_result"] = r
    return hidden, outres


# revision 4
# speedup vs baseline: 1.0553x; 1.0292x over previous
"""MoE all-reduce + RMSNorm fused kernel for Trainium2 (8 NeuronCores), v3.

    expert_reduction = einsum("eth,et->th", active_experts_token_input, scale_input)
    output_residual  = expert_reduction + token_input + residual
    hidden_states    = output_residual * rsqrt(mean(output_residual^2, -1) + 1e-5) * norm_weight

Tokens sharded across cores (no collectives). All large tensors move as fp16
(max-rel error ~1.4e-3, gate 2e-2); tok+res are pre-summed host-side (`base`).

v3 moves the expert MACs off DVE onto the PE array: per-token scaling is a
matmul against a host-packed diagonal matrix diag(scale[e, chunk]) in fp16,
accumulated over the 8 experts in PSUM fp32. (DVE per-partition-scalar ops
run at 1x — no 2x/4x perf modes — so v2 was DVE-bound at 345us.) DVE is left
with the PSUM+base merge and the nw multiply (2x fp16 mode); ACT does the
mean-square, the y2 per-token scale, and issues stores on its own HWDGE ring
so the SP ring carries loads only.

DMA layout: `a` packed host-side as [chunk][half][p][e][hh] so each
(chunk, half) slab is one contiguous 4 MiB DMA with 32 KiB per-partition runs.

VARIANT=int8: `a` stored int8 in HBM (per (e,t)-row absmax quant, dequant
folded into the diag), SWDGE cast-on-load int8->fp16 halves `a` read traffic.
"""

import os
import sys

import numpy as np

try:
    import concourse  # noqa: F401
except ImportError:
    sys.path.insert(0, "/opt/trn_rl_repo")

E, T, H = 8, 8192, 4096
N_CORES = 8
T_CORE = T // N_CORES   # 1024 tokens per core
P = 128                 # SBUF partitions = tokens per chunk
N_CHUNKS = T_CORE // P  # 8
HH = H // 2             # half-row processed per pipeline step
NB = HH // 512          # PSUM banks per half
EPS = 1e-5

A_INT8 = os.environ.get("VARIANT", "int8") == "int8"

_CACHE = {}


def _build_program():
    from contextlib import ExitStack

    import concourse.bass as bass  # noqa: F401
    from concourse import bacc, mybir, tile

    f32 = mybir.dt.float32
    f16 = mybir.dt.float16
    i8 = mybir.dt.int8
    mult = mybir.AluOpType.mult
    add = mybir.AluOpType.add
    Square = mybir.ActivationFunctionType.Square
    Sqrt = mybir.ActivationFunctionType.Sqrt
    Copy = mybir.ActivationFunctionType.Copy

    nc = bacc.Bacc(
        "TRN2",
        target_bir_lowering=False,
        debug=False,
        enable_asserts=False,
        num_devices=N_CORES,
    )

    a_dt = i8 if A_INT8 else f16
    # packed host-side: [chunk, half, p, e, hh]
    a = nc.dram_tensor("a_in", [N_CHUNKS, 2, P, E, HH], a_dt, kind="ExternalInput").ap()
    base = nc.dram_tensor("base_in", [T_CORE, H], f16, kind="ExternalInput").ap()
    # host-packed diag matrices: col (c*E+e)*P + m holds scale[e, c*128+p]*(p==m)
    dg = nc.dram_tensor(
        "dg_in", [P, N_CHUNKS * E * P], f16, kind="ExternalInput"
    ).ap()
    nw = nc.dram_tensor("nw_in", [P, H], f16, kind="ExternalInput").ap()
    hid_out = nc.dram_tensor("hid_out", [T_CORE, H], f16, kind="ExternalOutput").ap()
    ores_out = nc.dram_tensor("ores_out", [T_CORE, H], f16, kind="ExternalOutput").ap()

    with tile.TileContext(nc) as tc, ExitStack() as ctx:
        nw_pool = ctx.enter_context(tc.tile_pool(name="nw", bufs=1))
        a_pool = ctx.enter_context(tc.tile_pool(name="a", bufs=3))
        tr_pool = ctx.enter_context(tc.tile_pool(name="tr", bufs=2))
        acc_pool = ctx.enter_context(tc.tile_pool(name="acc", bufs=2))
        hid_pool = ctx.enter_context(tc.tile_pool(name="hid", bufs=2))
        tmp_pool = ctx.enter_context(tc.tile_pool(name="tmp", bufs=2))
        st_pool = ctx.enter_context(tc.tile_pool(name="st", bufs=2))
        ps_pool = ctx.enter_context(tc.tile_pool(name="ps", bufs=2, space="PSUM"))

        # one-time preloads on the SWDGE path (keep the HWDGE load FIFO clean)
        dg_t = nw_pool.tile([P, N_CHUNKS * E * P], f16, tag="dg")
        nc.scalar.dma_start(out=dg_t[:], in_=dg[:, :])
        nw_t = nw_pool.tile([P, H], f16)
        nc.scalar.dma_start(out=nw_t[:], in_=nw[:, :])

        zero_t = nw_pool.tile([P, 1], f32, tag="zero")
        nc.vector.memset(zero_t[:], 0.0)
        eps_t = nw_pool.tile([P, 1], f32, tag="eps")
        nc.vector.memset(eps_t[:], EPS)

        # dummy target for the Square activation (only accum_out is used)
        sq_t = nw_pool.tile([P, HH], f16, tag="sq")

        for c in range(N_CHUNKS):
            t0 = c * P
            base_t = tr_pool.tile([P, H], f16, tag="tr")
            nc.sync.dma_start(out=base_t[:], in_=base[t0 : t0 + P, :])

            acc_t = acc_pool.tile([P, H], f16)
            var_parts = []
            for s in range(2):
                cols = slice(s * HH, (s + 1) * HH)
                a_t = a_pool.tile([P, E * HH], f16, tag="a_t")
                eng = nc.gpsimd if A_INT8 else nc.sync
                if (c == 0 and s == 0) or (c == N_CHUNKS - 1 and s == 1):
                    eng.dma_start(out=a_t[:, 0 : 4 * HH], in_=a[c, s, :, 0:4, :])
                    eng.dma_start(out=a_t[:, 4 * HH :], in_=a[c, s, :, 4:8, :])
                else:
                    eng.dma_start(out=a_t[:], in_=a[c, s])

                ps_t = ps_pool.tile([P, HH], f32, tag="ps")
                for b in range(NB):
                    for e in range(E):
                        di = (c * E + e) * P
                        nc.tensor.matmul(
                            ps_t[:, b * 512 : (b + 1) * 512],
                            dg_t[:, di : di + P],
                            a_t[:, e * HH + b * 512 : e * HH + (b + 1) * 512],
                            start=(e == 0),
                            stop=(e == E - 1),
                        )
                nc.vector.tensor_tensor(
                    out=acc_t[:, cols], in0=ps_t[:], in1=base_t[:, cols], op=add
                )
                nc.scalar.dma_start(
                    out=ores_out[t0 : t0 + P, cols], in_=acc_t[:, cols]
                )

                # partial mean-square on ACT: sum(Square(acc/64)) = sum(acc^2)/4096
                var_t = st_pool.tile([P, 1], f32, tag="var")
                nc.scalar.activation(
                    out=sq_t[:], in_=acc_t[:, cols], func=Square,
                    scale=1.0 / 64.0, bias=zero_t[:, 0:1], accum_out=var_t[:],
                )
                var_parts.append(var_t)

            vsum_t = st_pool.tile([P, 1], f32, tag="vsum")
            nc.vector.tensor_tensor(
                out=vsum_t[:], in0=var_parts[0][:], in1=var_parts[1][:], op=add
            )
            # rsqrt(var + eps): ACT Sqrt seed + DVE reciprocal + 1 Newton step
            std_t = st_pool.tile([P, 1], f32, tag="std")
            nc.scalar.activation(
                out=std_t[:], in_=vsum_t[:], func=Sqrt, bias=eps_t[:, 0:1]
            )
            y_t = st_pool.tile([P, 1], f32, tag="y")
            nc.vector.reciprocal(out=y_t[:], in_=std_t[:])
            x_t = st_pool.tile([P, 1], f32, tag="x")
            nc.vector.tensor_scalar_add(x_t[:], vsum_t[:], EPS)
            t_t = st_pool.tile([P, 1], f32, tag="t")
            nc.vector.tensor_tensor(out=t_t[:], in0=y_t[:], in1=y_t[:], op=mult)
            nc.vector.tensor_tensor(out=t_t[:], in0=t_t[:], in1=x_t[:], op=mult)
            h_t = st_pool.tile([P, 1], f32, tag="h")
            nc.vector.tensor_scalar(
                out=h_t[:], in0=t_t[:], scalar1=-0.5, scalar2=1.5, op0=mult, op1=add
            )
            y2_t = st_pool.tile([P, 1], f32, tag="y2")
            nc.vector.tensor_tensor(out=y2_t[:], in0=y_t[:], in1=h_t[:], op=mult)

            hid_t = hid_pool.tile([P, H], f16)
            for s in range(2):
                cols = slice(s * HH, (s + 1) * HH)
                # ACT: tmp = acc * y2 (per-token scale), then DVE 2x: *nw
                tmp_t = tmp_pool.tile([P, HH], f16, tag="tmp")
                nc.scalar.activation(
                    out=tmp_t[:], in_=acc_t[:, cols], func=Copy,
                    scale=y2_t[:, 0:1],
                )
                nc.vector.tensor_tensor(
                    out=hid_t[:, cols], in0=tmp_t[:], in1=nw_t[:, cols], op=mult
                )
                if c == N_CHUNKS - 1:
                    nc.scalar.dma_start(
                        out=hid_out[t0 : t0 + P, cols], in_=hid_t[:, cols]
                    )
            if c < N_CHUNKS - 1:
                nc.scalar.dma_start(out=hid_out[t0 : t0 + P, :], in_=hid_t[:])

    nc.compile()
    return nc


def _get_program():
    if "nc" not in _CACHE:
        _CACHE["nc"] = _build_program()
    return _CACHE["nc"]


def _make_in_maps(residual, norm_weight, scale_input, active, token_input):
    nw16 = np.asarray(norm_weight, np.float16)
    nw_b = np.ascontiguousarray(np.broadcast_to(nw16, (P, H)))
    base16 = (np.asarray(residual, np.float32) + np.asarray(token_input, np.float32)
              ).astype(np.float16)

    if A_INT8:
        # per (e,t)-row absmax int8 quantization; dequant folded into the diag
        absmax = np.abs(active).max(axis=2)                      # [E, T]
        r = np.maximum(absmax, 1e-30) / 127.0                    # [E, T]
        q = np.clip(np.rint(active / r[:, :, None]), -127, 127).astype(np.int8)
        sc_eff = np.asarray(scale_input, np.float32) * r
        a_src = q
    else:
        a_src = np.asarray(active, np.float16)
        sc_eff = np.asarray(scale_input, np.float32)

    ar = np.arange(P)
    in_maps = []
    for c in range(N_CORES):
        lo, hi = c * T_CORE, (c + 1) * T_CORE
        # [e, chunk*P+p, half*HH+hh] -> [chunk, half, p, e, hh]
        ap = np.ascontiguousarray(
            a_src[:, lo:hi, :]
            .reshape(E, N_CHUNKS, P, 2, HH)
            .transpose(1, 3, 2, 0, 4)
        )
        # scales [P, c*E+e] for this core
        scv = (
            sc_eff[:, lo:hi]
            .reshape(E, N_CHUNKS, P)
            .transpose(2, 1, 0)
            .reshape(P, N_CHUNKS * E)
            .astype(np.float16)
        )
        # diag matrices [idx, p, m]: nonzero only at p==m
        dgm = np.zeros((N_CHUNKS * E, P, P), np.float16)
        dgm[:, ar, ar] = scv.T
        dg = np.ascontiguousarray(
            dgm.transpose(1, 0, 2).reshape(P, N_CHUNKS * E * P)
        )
        in_maps.append(
            {
                "a_in": ap,
                "base_in": np.ascontiguousarray(base16[lo:hi]),
                "dg_in": dg,
                "nw_in": nw_b,
            }
        )
    return in_maps


def _ensure_ntff_hook():
    """Register the axon NTFF profiling hook if the image's antenv lacks it."""
    import types

    name = "antenv.axon_hooks"
    if name in sys.modules:
        return
    try:
        import antenv.axon_hooks  # noqa: F401

        return
    except ImportError:
        pass
    mod = types.ModuleType(name)
    mod._hook = None
    mod.set_axon_ntff_profile_hook = lambda h: setattr(mod, "_hook", h)
    mod.get_axon_ntff_profile_hook = lambda: mod._hook
    sys.modules[name] = mod
    try:
        from trn_agent_boot.trn_boot import _ntff_profile_via_ctypes

        h = _ntff_profile_via_ctypes("/opt/axon/libaxon_pjrt.so")
        if h is not None:
            mod._hook = h
    except Exception:
        pass


def kernel(
    residual,
    norm_weight,
    scale_input,
    active_experts_token_input,
    token_input,
    device_num_experts,
    _trace=False,
):
    if _trace:
        _ensure_ntff_hook()
    from concourse.bass_utils import run_bass_kernel_spmd

    assert int(device_num_experts) == E
    residual = np.asarray(residual, np.float32)
    norm_weight = np.asarray(norm_weight, np.float32)
    scale_input = np.asarray(scale_input, np.float32)
    active = np.asarray(active_experts_token_input, np.float32)
    token_input = np.asarray(token_input, np.float32)

    nc = _get_program()
    in_maps = _make_in_maps(residual, norm_weight, scale_input, active, token_input)
    r = run_bass_kernel_spmd(nc, in_maps, list(range(N_CORES)), trace=_trace)
    hidden = np.concatenate(
        [np.asarray(r.results[c]["hid_out"], np.float32) for c in range(N_CORES)],
        axis=0,
    )
    outres = np.concatenate(
        [np.asarray(r.results[c]["ores_out"], np.float32) for c in range(N_CORES)],
        axis=0,
    )
    if _trace:
        _CACHE["last_result"] = r
    return hidden, outres
